# revision 1
# baseline (speedup 1.0000x reference)
"""Causal multi-head attention with RoPE on 8 TRN2 NeuronCores.

Problem: BS=2, SEQ=2048, DIM=2048, NH=16, HD=128 (fp32).
Sharding: core = b*4 + g  (b = batch, g = head-group of 4 heads).
Each core computes q/k/v for its 4 heads from its batch's x, applies RoPE,
causal attention, and a partial output projection through its 512-row slice
of wo. The host sums the 4 per-group partials per batch.

Per-core layouts (partition dim first):
  xT [d, s] (host pre-tiled per s-block); QT/KT [hd, s] per head; V [s, 4*hd];
  scoresT [j, i]; exp; PV -> outT [hd, i]; rowsum via ones-matmul;
  normalize outT via gpsimd partition_broadcast of 1/rowsum;
  wo with lhsT=outT tiles -> final [i, d] partial.

Matmuls in float32r (full PE rate at moving-dim 512, ~10-bit mantissa) or
bf16 (MM_DT flag). RoPE pair-swap via DVE stream_shuffle (mask i^1).
Host pre-tiles all DRAM operands so each DMA is one trigger with
per-partition-contiguous descriptors.
"""
import math
import numpy as np
from contextlib import ExitStack

import concourse.bass as bass
import concourse.bacc as bacc
import concourse.tile as tile
import concourse.mybir as mybir
from concourse import bass_utils

F32 = mybir.dt.float32
F32R = mybir.dt.float32r
BF16 = mybir.dt.bfloat16
AF = mybir.ActivationFunctionType

SEQ = 2048
DIM = 2048
HD = 128
MG = 512                       # per-core head width (4 heads x 128)
ND = DIM // 128                # 16 d-tiles
PAIR_SWAP = [i ^ 1 for i in range(32)]

MM_DT = F32R                   # matmul operand dtype: F32R or BF16

_CACHED = {}


def build_nc(seq=SEQ, mm_dt=None):
    mm_dt = mm_dt or MM_DT
    NSB = seq // 512
    NST = seq // 128
    nc = bacc.Bacc("TRN2", target_bir_lowering=False, debug=False)

    x_d = nc.dram_tensor("x_t", [NSB, 128, ND, 512], mm_dt, kind="ExternalInput")
    wq_d = nc.dram_tensor("wq_t", [128, ND, 512], mm_dt, kind="ExternalInput")
    wk_d = nc.dram_tensor("wk_t", [128, ND, 512], mm_dt, kind="ExternalInput")
    wv_d = nc.dram_tensor("wv_t", [128, ND, 512], mm_dt, kind="ExternalInput")
    wo_d = nc.dram_tensor("wo_t", [128, 4, DIM], mm_dt, kind="ExternalInput")
    trq_d = nc.dram_tensor("trigq", [128, 2, seq], F32, kind="ExternalInput")
    trk_d = nc.dram_tensor("trigk", [128, 2, seq], F32, kind="ExternalInput")
    msk_d = nc.dram_tensor("masks_t", [128, 4, 512], BF16, kind="ExternalInput")
    onc_d = nc.dram_tensor("ones_sq", [128, 128], mm_dt, kind="ExternalInput")
    out_d = nc.dram_tensor("out", [seq, DIM], F32, kind="ExternalOutput")

    with tile.TileContext(nc) as tc, ExitStack() as ctx:
        persist = ctx.enter_context(tc.tile_pool(name="persist", bufs=1))
        ktr = [persist.tile([128, seq], mm_dt, tag=f"ktr{h}", name=f"ktr{h}")
               for h in range(4)]
        v_sb = persist.tile([128, NST, MG], mm_dt, tag="v")
        ones_sq = persist.tile([128, 128], mm_dt, tag="onesq")
        nc.sync.dma_start(ones_sq[:], onc_d.ap())
        msk = persist.tile([128, 4, 512], BF16, tag="masks")
        nc.sync.dma_start(msk[:], msk_d.ap())
        qtrp = ctx.enter_context(tc.tile_pool(name="qtrp", bufs=1))
        qtr = [qtrp.tile([128, seq], mm_dt, tag=f"qtr{h}", name=f"qtr{h}")
               for h in range(4)]

        def rope(ropet, prawp, psum_t, out_slice, trig_t):
            """out = p*cos + shuffle(p)*sin; ACT-copy psum->sbuf first so the
            PSUM bank frees without waiting the DVE chain."""
            praw = prawp.tile([128, 512], F32, tag="praw", name="praw")
            nc.scalar.copy(praw[:], psum_t[:])
            shuf = ropet.tile([128, 512], F32, tag="shuf", name="shuf")
            nc.vector.stream_shuffle(shuf[:], praw[:], PAIR_SWAP)
            t1 = ropet.tile([128, 512], F32, tag="t1", name="t1")
            nc.vector.tensor_mul(t1[:], praw[:], trig_t[:, 0, :])
            nc.vector.tensor_mul(shuf[:], shuf[:], trig_t[:, 1, :])
            nc.vector.tensor_add(out_slice, t1[:], shuf[:])

        # ---- phases 0+1: shared streaming pools ----
        shared = ExitStack()
        trigp = shared.enter_context(tc.tile_pool(name="trigp", bufs=2))
        prawp = shared.enter_context(tc.tile_pool(name="prawp", bufs=4))
        ropet = shared.enter_context(tc.tile_pool(name="ropep", bufs=1))
        xpool = shared.enter_context(tc.tile_pool(name="xpool", bufs=3))
        wvpool = shared.enter_context(tc.tile_pool(name="wvpool", bufs=2))

        def load_w_quarters(dst, dram_ap):
            for qtr4 in range(4):
                nc.sync.dma_start(dst[:, qtr4 * 4:(qtr4 + 1) * 4, :],
                                  dram_ap[:, qtr4 * 4:(qtr4 + 1) * 4, :])

        # ---- phase 0: K + V in one x sweep; d-outer; 8 psum banks ----
        with tc.tile_pool(name="p0w", bufs=1) as p0w, \
             tc.tile_pool(name="p0psk", bufs=1, space="PSUM") as p0psk, \
             tc.tile_pool(name="p0psv", bufs=1, space="PSUM") as p0psv:
            trig0, rope0, p0x = trigp, ropet, xpool
            wk_sb = p0w.tile([128, ND, MG], mm_dt, tag="wk")
            nc.sync.dma_start(wk_sb[:, 0:4, :], wk_d.ap()[:, 0:4, :])
            for sb in range(NSB):
                s0 = sb * 512
                tg = trig0.tile([128, 2, 512], F32, tag="tg", name="tgk")
                nc.sync.dma_start(tg[:], trk_d.ap()[:, :, s0:s0 + 512])
                ps_k = [p0psk.tile([128, 512], F32, tag=f"psk{m}", name=f"psk{m}")
                        for m in range(4)]
                ps_v = [p0psv.tile([128, 512], F32, tag=f"psv{st}", name=f"psv{st}")
                        for st in range(4)]
                xh, wvh = [], []
                for q4 in range(4):
                    xt = p0x.tile([128, 4, 512], mm_dt, tag="x", name="xt")
                    nc.sync.dma_start(xt[:], x_d.ap()[sb, :, q4 * 4:q4 * 4 + 4, :])
                    xh.append(xt)
                    wt = wvpool.tile([128, 4, 512], mm_dt, tag="wvh", name="wvh")
                    nc.sync.dma_start(wt[:], wv_d.ap()[:, q4 * 4:q4 * 4 + 4, :])
                    wvh.append(wt)
                    if sb == 0 and q4 == 0:
                        for wq4 in range(1, 4):
                            nc.sync.dma_start(
                                wk_sb[:, wq4 * 4:(wq4 + 1) * 4, :],
                                wk_d.ap()[:, wq4 * 4:(wq4 + 1) * 4, :])
                for dt in range(ND):
                    xs = xh[dt // 4][:, dt % 4, :]
                    for m in range(4):
                        nc.tensor.matmul(
                            ps_k[m][:], wk_sb[:, dt, m * 128:(m + 1) * 128],
                            xs, start=(dt == 0), stop=(dt == ND - 1))
                    for st in range(4):
                        nc.tensor.matmul(
                            ps_v[st][:], xs[:, st * 128:(st + 1) * 128],
                            wvh[dt // 4][:, dt % 4, :],
                            start=(dt == 0), stop=(dt == ND - 1))
                for m in range(4):
                    rope(rope0, prawp, ps_k[m], ktr[m][:, s0:s0 + 512], tg)
                for st in range(4):
                    nc.scalar.copy(v_sb[:, sb * 4 + st, :], ps_v[st][:])

        # ---- phase 1: Q sweep; d-outer; 4 psum banks ----
        with tc.tile_pool(name="p1w", bufs=1) as p1w, \
             tc.tile_pool(name="p1ps", bufs=2, space="PSUM") as p1ps:
            trig1, rope1, p1x = trigp, ropet, xpool
            wq_sb = p1w.tile([128, ND, MG], mm_dt, tag="wq")
            load_w_quarters(wq_sb, wq_d.ap())
            for sb in range(NSB):
                s0 = sb * 512
                tg = trig1.tile([128, 2, 512], F32, tag="tg", name="tgq")
                nc.sync.dma_start(tg[:], trq_d.ap()[:, :, s0:s0 + 512])
                ps_q = [p1ps.tile([128, 512], F32, tag=f"psq{m}", name=f"psq{m}")
                        for m in range(4)]
                xh = []
                for q4 in range(4):
                    xt = p1x.tile([128, 4, 512], mm_dt, tag="x", name="xt")
                    nc.sync.dma_start(xt[:], x_d.ap()[sb, :, q4 * 4:q4 * 4 + 4, :])
                    xh.append(xt)
                for dt in range(ND):
                    xs = xh[dt // 4][:, dt % 4, :]
                    for m in range(4):
                        nc.tensor.matmul(
                            ps_q[m][:], wq_sb[:, dt, m * 128:(m + 1) * 128],
                            xs, start=(dt == 0), stop=(dt == ND - 1))
                for m in range(4):
                    rope(rope1, prawp, ps_q[m], qtr[m][:, s0:s0 + 512], tg)

        shared.close()

        # ---- phase 2: attention + wo ----
        with tc.tile_pool(name="p2w", bufs=1) as p2w, \
             tc.tile_pool(name="ep", bufs=6) as ep, \
             tc.tile_pool(name="etmp", bufs=2) as etmp, \
             tc.tile_pool(name="otn", bufs=2) as otn, \
             tc.tile_pool(name="bcp", bufs=1) as bcp, \
             tc.tile_pool(name="wout", bufs=3) as wout, \
             tc.tile_pool(name="ps_s", bufs=3, space="PSUM") as ps_s, \
             tc.tile_pool(name="ps_o", bufs=2, space="PSUM") as ps_o, \
             tc.tile_pool(name="ps_r", bufs=1, space="PSUM") as ps_r, \
             tc.tile_pool(name="ps_w", bufs=2, space="PSUM") as ps_w:
            wo_sb = p2w.tile([128, 4, DIM], mm_dt, tag="wo")
            for c in range(4):
                nc.sync.dma_start(wo_sb[:, c, :], wo_d.ap()[:, c, :])

            def wo_block(ibp, it):
                i0p = ibp * 512
                for dblk in range(4):
                    pw = ps_w.tile([128, 512], F32, tag="w", name="w")
                    for c in range(4):
                        nc.tensor.matmul(
                            pw[:], prev_outn[0][:, c, it * 128:(it + 1) * 128],
                            wo_sb[:, c, dblk * 512:(dblk + 1) * 512],
                            start=(c == 0), stop=(c == 3))
                    ow = wout.tile([128, 512], F32, tag="ow", name="ow")
                    nc.vector.tensor_copy(ow[:], pw[:])
                    nc.sync.dma_start(
                        out_d.ap()[i0p + it * 128:i0p + (it + 1) * 128,
                                   dblk * 512:(dblk + 1) * 512], ow[:])

            prev_outn = [None]
            for ib in range(NSB):
                i0 = ib * 512
                nj = 4 * ib + 4
                outn = otn.tile([128, 4, 512], mm_dt, tag="outn", name="outn")
                rs4 = bcp.tile([128, 4, 512], F32, tag="rs4", name="rs4")
                for h in range(4):
                    po = ps_o.tile([128, 512], F32, tag="pv", name="pv")
                    prbc = ps_r.tile([128, 512], F32, tag="rs", name="rs")
                    for tj in range(nj):
                        r = tj - 4 * ib
                        i_lo, nw = (256, 256) if r >= 2 else (0, 512)
                        pscr = ps_s.tile([128, 512], F32, tag="sc", name="sc")
                        nc.tensor.matmul(
                            pscr[:, 0:nw], ktr[h][:, tj * 128:(tj + 1) * 128],
                            qtr[h][:, i0 + i_lo:i0 + i_lo + nw],
                            start=True, stop=True)
                        e_t = ep.tile([128, 512], mm_dt, tag="e", name="e")
                        if r >= 0:                # diagonal-overlap tile
                            et = etmp.tile([128, 512], F32, tag="etmp", name="et")
                            nc.scalar.activation(et[:, 0:nw], pscr[:, 0:nw], AF.Exp)
                            nc.vector.tensor_mul(e_t[:, 0:nw], et[:, 0:nw],
                                                 msk[:, r, i_lo:512])
                        else:
                            nc.scalar.activation(e_t[:], pscr[:], AF.Exp)
                        nc.tensor.matmul(
                            po[:, i_lo:i_lo + nw],
                            v_sb[:, tj, h * 128:(h + 1) * 128], e_t[:, 0:nw],
                            start=(tj == 0), stop=(tj == nj - 1))
                        nc.tensor.matmul(
                            prbc[:, i_lo:i_lo + nw], ones_sq[:], e_t[:, 0:nw],
                            start=(tj == 0), stop=(tj == nj - 1))
                    nc.vector.tensor_copy(rs4[:, h, :], prbc[:])
                    nc.vector.tensor_copy(outn[:, h, :], po[:])
                    if prev_outn[0] is not None:
                        wo_block(ib - 1, h)
                rbc = bcp.tile([128, 4, 512], F32, tag="rbc", name="rbc")
                nc.scalar.activation(rbc[:], rs4[:], AF.Ln)
                nc.scalar.activation(rbc[:], rbc[:], AF.Exp, scale=-1.0)
                for h in range(4):
                    nc.vector.tensor_mul(outn[:, h, :], outn[:, h, :], rbc[:, h, :])
                prev_outn[0] = outn

            for it in range(4):
                wo_block(NSB - 1, it)

    nc.compile()
    return nc


def _host_prep(x, freqs_cos, freqs_sin, wq, wk, wv, wo, mm_dt=None, seq=SEQ):
    """Build the 8 per-core input maps with pre-tiled layouts."""
    mm_dt = mm_dt or MM_DT
    npdt = mybir.dt.np(mm_dt)
    bs = x.shape[0]
    NSB = seq // 512
    scale = np.float32(1.0 / math.sqrt(HD))

    cos_e = np.repeat(np.asarray(freqs_cos).T, 2, axis=0).astype(np.float32)
    sin_raw = np.repeat(np.asarray(freqs_sin).T, 2, axis=0).astype(np.float32)
    sin_e = sin_raw.copy()
    sin_e[0::2] = -sin_raw[0::2]      # out[2i] = q[2i]cos - q[2i+1]sin
    trigk = np.ascontiguousarray(np.stack([cos_e, sin_e], axis=1))
    trigq = np.ascontiguousarray(trigk * scale)

    jr = np.arange(128)[:, None]
    ir = np.arange(512)[None, :]
    masks = np.zeros((4, 128, 512), dtype=np.float32)
    for r in range(4):
        masks[r] = (128 * r + jr <= ir).astype(np.float32)
    masks_t = np.ascontiguousarray(masks.transpose(1, 0, 2)).astype(mybir.dt.np(BF16))

    ones_sq = np.ones((128, 128), npdt)

    def wtile(w):  # [DIM, 512] -> [128, 16, 512]
        return np.ascontiguousarray(
            np.asarray(w).reshape(ND, 128, MG).transpose(1, 0, 2)).astype(npdt)

    x_t = []
    for b in range(bs):
        xt = np.asarray(x[b]).reshape(NSB, 512, ND, 128).transpose(0, 3, 2, 1)
        x_t.append(np.ascontiguousarray(xt).astype(npdt))

    in_maps = []
    for core in range(8):
        b, g = divmod(core, 4)
        b = min(b, bs - 1)
        wo_g = np.asarray(wo)[g * MG:(g + 1) * MG, :]
        in_maps.append({
            "x_t": x_t[b],
            "wq_t": wtile(np.asarray(wq)[:, g * MG:(g + 1) * MG]),
            "wk_t": wtile(np.asarray(wk)[:, g * MG:(g + 1) * MG]),
            "wv_t": wtile(np.asarray(wv)[:, g * MG:(g + 1) * MG]),
            "wo_t": np.ascontiguousarray(
                wo_g.reshape(4, 128, DIM).transpose(1, 0, 2)).astype(npdt),
            "trigq": trigq, "trigk": trigk,
            "masks_t": masks_t, "ones_sq": ones_sq,
        })
    return in_maps


def kernel(x, freqs_cos, freqs_sin, mask, wq, wk, wv, wo, _trace=False):
    x = np.asarray(x, dtype=np.float32)
    in_maps = _host_prep(x, np.asarray(freqs_cos), np.asarray(freqs_sin),
                         np.asarray(wq), np.asarray(wk), np.asarray(wv),
                         np.asarray(wo))
    if "nc" not in _CACHED:
        _CACHED["nc"] = build_nc()
    nc = _CACHED["nc"]
    res = bass_utils.run_bass_kernel_spmd(nc, in_maps, core_ids=list(range(8)),
                                          trace=_trace)
    if _trace:
        _CACHED["last_exec_time_ns"] = res.exec_time_ns
        _CACHED["last_trace"] = res.instructions_and_trace
    bs = x.shape[0]
    out = np.zeros((bs, SEQ, DIM), dtype=np.float32)
    for core in range(8):
        out[core // 4] += res.results[core]["out"]
    return out

